# revision 5
# baseline (speedup 1.0000x reference)
"""Trainium2 kernel for nn_COSSIMMLP (gnn_message_passing).

reference semantics:
    src = prop_state[b, mask[...,0]]; dst = prop_state[b, mask[...,1]]
    vals = sigmoid(cossim(src, dst))          # [B, E]
    adj[b, i, j] = vals; adj[b, j, i] = vals  # dense [B, N, N]

Every scatter write at position (r, c) carries the identical value
sigmoid(cos(s_r, s_c)), and adj is exactly symmetric with zeros at
non-edge positions.  The device therefore computes only the folded
half-gram G = (4*S_hat)(4*S_hat)^T in fp8 (so each slab entry holds
16*cos) and ships it back raw; the host gathers the ~E edge entries,
applies the exact sigmoid to just those, and scatters them into a
zeroed dense adjacency.  Non-edges are exact zeros, so no mask tensor
ever crosses the DMA, and no engine touches a 4M-entry sigmoid.

8 cores = 4 batches x 2 LHS-tile-halves, node order rolled per core by
2048*h so one SPMD program serves all cores.  In 128-row tile
coordinates the core owning LHS tiles m=0..15 computes gram blocks
(m, m+d) for ring distance d=0..15 as a [2048, 2048] slab plus d=16 as
a separate [128, 16*128] strip (strip layout: partition p, col t*128+c
holds gram row t*128+p, ring-16 col c; computed redundantly by both
cores of a pair).  fp8 e4m3 holds 16*cos to ~2% which perturbs
sigmoid(cos) by only ~6e-4 relative (cos ~ N(0,1/256) for D=256), far
inside the 2e-2 gate.

Engine budget per core: PE streams 16*2048+2048 DoubleRow fp8 columns
(~15us at 2.4GHz incl. weight loads), PSUM->SBUF fp8 cast-copies are
the critical path and are split ACT:DVE ~ 1088:960 columns per m-tile
(one big instruction each, ~1.17us/tile), DMA moves 1MB in + 4.5MB out
(~15.5us).  Input is host-normalized fp8 in partition-contiguous
layout, loaded as two HWDGE pieces (sync + gpsimd queues) so the first
matmul starts as soon as cols 0..2304 land.
"""

import numpy as np
import ml_dtypes

B, N, D, E = 4, 4096, 256, 131072
P = 128              # partitions
MT = 16              # LHS tiles per core (2048 rows)
ROWS = MT * P        # 2048
MCOLS = 16 * P       # 2048 main cols per slab row-tile (ring distance 0..15)
COLS = 17 * P        # 2176 incl. the d=16 strip
EPS = 1e-8
ACT_COLS = 1088      # ACT/DVE split of each 2048-col copy (balances
DVE_COLS = MCOLS - ACT_COLS  # 0.833ns/col+260ns vs 1.042ns/col+158ns)

_prog = None


def _build_program():
    import concourse.tile as tile
    from concourse import bacc, mybir

    f32 = mybir.dt.float32
    fp8 = mybir.dt.float8e4
    DR = mybir.MatmulPerfMode.DoubleRow

    nc = bacc.Bacc("TRN2", target_bir_lowering=False, debug=False)
    # st[p, kt*N + n] = 4*s_hat[node n, dim kt*128+p]: one contiguous 8KB
    # line per partition, DoubleRow k-major
    st_in = nc.dram_tensor("st", [P, 2 * N], fp8, kind="ExternalInput")
    outb = nc.dram_tensor("outb", [ROWS, MCOLS], fp8, kind="ExternalOutput")
    outc = nc.dram_tensor("outc", [P, MT * P], fp8, kind="ExternalOutput")

    st_r = st_in.rearrange("p (kt n) -> p kt n", kt=2)

    with tile.TileContext(nc) as tc:
        with (
            tc.tile_pool(name="const", bufs=1) as cpool,
            tc.tile_pool(name="outp", bufs=3) as outp,
        ):
            st = cpool.tile([P, 2, N], fp8)
            # piece A on the sync HWDGE queue unblocks m=0; piece B rides
            # the otherwise idle gpsimd queue
            nc.sync.dma_start(out=st[:, :, 0:2304], in_=st_r[:, :, 0:2304])
            nc.gpsimd.dma_start(out=st[:, :, 2304:N], in_=st_r[:, :, 2304:N])

            with tc.tile_pool(name="mmps", bufs=2, space="PSUM") as mmps:
                for m in range(MT):
                    base = m * P
                    lhs = st[:, :, base : base + P]
                    ot = outp.tile([P, MCOLS], fp8, tag="ot")
                    ps = mmps.tile([P, MCOLS], f32, tag="ps")
                    for q in range(4):
                        c0 = q * 512
                        nc.tensor.matmul(
                            ps[:, c0 : c0 + 512],
                            lhsT=lhs,
                            rhs=st[:, :, base + c0 : base + c0 + 512],
                            perf_mode=DR,
                            start=True,
                            stop=True,
                        )
                    nc.scalar.copy(out=ot[:, :ACT_COLS], in_=ps[:, :ACT_COLS])
                    nc.vector.tensor_copy(
                        out=ot[:, ACT_COLS:], in_=ps[:, ACT_COLS:]
                    )
                    nc.sync.dma_start(out=outb[base : base + P, :], in_=ot[:])

            # d=16 tail strip: 16 small matmuls into one 4-bank PSUM tile
            with tc.tile_pool(name="tlps", bufs=1, space="PSUM") as tlps:
                otc = outp.tile([P, MT * P], fp8, tag="otc")
                pst = tlps.tile([P, MT * P], f32, tag="pst")
                for m in range(MT):
                    base = m * P
                    nc.tensor.matmul(
                        pst[:, base : base + P],
                        lhsT=st[:, :, base : base + P],
                        rhs=st[:, :, base + MCOLS : base + COLS],
                        perf_mode=DR,
                        start=True,
                        stop=True,
                    )
                nc.scalar.copy(out=otc[:, :ACT_COLS], in_=pst[:, :ACT_COLS])
                nc.vector.tensor_copy(
                    out=otc[:, ACT_COLS:], in_=pst[:, ACT_COLS:]
                )
                nc.sync.dma_start(out=outc[:, :], in_=otc[:])

    nc.compile()
    return nc


def _host_prep(prop_state, mask):
    prop = np.asarray(prop_state, dtype=np.float32)
    nrm = np.sqrt(np.einsum("bnd,bnd->bn", prop, prop))
    shat4 = prop * (4.0 / np.maximum(nrm, EPS))[..., None]
    shat4 = shat4.astype(ml_dtypes.float8_e4m3)  # [B, N, D]

    in_maps = []
    for c in range(8):
        b, h = divmod(c, 2)
        r = ROWS * h
        rolled = shat4[b] if r == 0 else np.roll(shat4[b], -r, axis=0)
        # [N, D] -> [P, 2*N] partition-contiguous DoubleRow k-major
        st = np.ascontiguousarray(
            rolled.T.reshape(2, P, N).transpose(1, 0, 2).reshape(P, 2 * N)
        )
        in_maps.append({"st": st})
    return in_maps


def _assemble(results, mask):
    mk = np.asarray(mask)
    out = np.zeros((B, N, N), dtype=np.float32)
    for b in range(B):
        i = mk[b, :, 0].astype(np.int64)
        j = mk[b, :, 1].astype(np.int64)
        mains = [results[2 * b]["outb"], results[2 * b + 1]["outb"]]
        strips = [results[2 * b]["outc"], results[2 * b + 1]["outc"]]
        val = np.empty(E, dtype=np.float64)
        found = np.zeros(E, dtype=bool)
        for x, y in ((i, j), (j, i)):
            for h in (0, 1):
                xr = (x - ROWS * h) % N
                yr = (y - ROWS * h) % N
                cc = yr - (xr >> 7 << 7)
                sel = ~found & (xr < ROWS)
                okm = sel & (cc >= 0) & (cc < MCOLS)
                idx = np.nonzero(okm)[0]
                if idx.size:
                    val[idx] = mains[h][xr[idx], cc[idx]].astype(np.float64)
                    found[idx] = True
                oks = sel & (cc >= MCOLS) & (cc < COLS)
                idx = np.nonzero(oks)[0]
                if idx.size:
                    xi = xr[idx]
                    val[idx] = strips[h][
                        xi & 127, (xi >> 7 << 7) + cc[idx] - MCOLS
                    ].astype(np.float64)
                    found[idx] = True
        assert found.all()
        v = 1.0 / (1.0 + np.exp(-val / 16.0))
        v[i == j] = 0.7310585786300049  # sigmoid(1): self-cossim is exactly 1
        v = v.astype(np.float32)
        out[b, i, j] = v
        out[b, j, i] = v
    return out


def kernel(prop_state, mask):
    from concourse.bass_utils import run_bass_kernel_spmd

    global _prog
    if _prog is None:
        _prog = _build_program()
    in_maps = _host_prep(prop_state, mask)
    res = run_bass_kernel_spmd(_prog, in_maps, core_ids=list(range(8)))
    return _assemble(res.results, mask)


# revision 9
# speedup vs baseline: 1.0731x; 1.0731x over previous
"""Trainium2 kernel for nn_COSSIMMLP (gnn_message_passing).

reference semantics:
    src = prop_state[b, mask[...,0]]; dst = prop_state[b, mask[...,1]]
    vals = sigmoid(cossim(src, dst))          # [B, E]
    adj[b, i, j] = vals; adj[b, j, i] = vals  # dense [B, N, N]

Every scatter write at position (r, c) carries the identical value
sigmoid(cos(s_r, s_c)), and adj is exactly symmetric with zeros at
non-edge positions.  The device therefore computes only the folded
half-gram G = (4*S_hat)(4*S_hat)^T in fp8 (so each slab entry holds
16*cos) and ships it back raw; the host gathers the ~E edge entries,
applies the exact sigmoid to just those, and scatters them into a
zeroed dense adjacency.  Non-edges are exact zeros, so no mask tensor
ever crosses the DMA, and no engine touches a 4M-entry sigmoid.

8 cores = 4 batches x 2 LHS-tile-halves, node order rolled per core by
2048*h so one SPMD program serves all cores.  In 128-row tile
coordinates the core owning LHS tiles m=0..15 computes gram blocks
(m, m+d) for ring distance d=0..15 as a [2048, 2048] slab plus d=16 as
a separate [128, 16*128] strip (strip layout: partition p, col t*128+c
holds gram row t*128+p, ring-16 col c; computed redundantly by both
cores of a pair).  fp8 e4m3 holds 16*cos to ~2% which perturbs
sigmoid(cos) by only ~6e-4 relative (cos ~ N(0,1/256) for D=256), far
inside the 2e-2 gate.

Engine budget per core: PE streams 16*2048+2048 DoubleRow fp8 columns
(~15us at 2.4GHz incl. weight loads), PSUM->SBUF fp8 cast-copies are
the critical path.  The tile framework serializes two engines that
write disjoint slices of the SAME SBUF tile, so each [128,2048] tile
is copied whole by ONE engine, alternating: ACT takes even tiles plus
the d=16 strip (9 x 1.97us), DVE takes odd tiles (8 x 2.29us) —
~18.3us pipelined against the matmuls and the 17 output DMAs (~15.5us
of wire).  Input is host-normalized fp8 in partition-contiguous
layout, loaded as three pieces on the sync/vector/gpsimd queues in
parallel so the m=0 matmuls start as early as possible.
"""

import numpy as np
import ml_dtypes

B, N, D, E = 4, 4096, 256, 131072
P = 128              # partitions
MT = 16              # LHS tiles per core (2048 rows)
ROWS = MT * P        # 2048
MCOLS = 16 * P       # 2048 main cols per slab row-tile (ring distance 0..15)
COLS = 17 * P        # 2176 incl. the d=16 strip
EPS = 1e-8
ACT_COLS = 1088      # ACT/DVE split of each 2048-col copy (balances
DVE_COLS = MCOLS - ACT_COLS  # 0.833ns/col+260ns vs 1.042ns/col+158ns)

_prog = None


def _build_program():
    import concourse.tile as tile
    from concourse import bacc, mybir

    f32 = mybir.dt.float32
    fp8 = mybir.dt.float8e4
    DR = mybir.MatmulPerfMode.DoubleRow

    nc = bacc.Bacc("TRN2", target_bir_lowering=False, debug=False)
    # st[p, kt*N + n] = 4*s_hat[node n, dim kt*128+p]: one contiguous 8KB
    # line per partition, DoubleRow k-major
    st_in = nc.dram_tensor("st", [P, 2 * N], fp8, kind="ExternalInput")
    outb = nc.dram_tensor("outb", [ROWS, MCOLS], fp8, kind="ExternalOutput")
    outc = nc.dram_tensor("outc", [P, MT * P], fp8, kind="ExternalOutput")

    st_r = st_in.rearrange("p (kt n) -> p kt n", kt=2)

    with tile.TileContext(nc) as tc:
        with (
            tc.tile_pool(name="const", bufs=1) as cpool,
            tc.tile_pool(name="outp", bufs=3) as outp,
        ):
            st = cpool.tile([P, 2, N], fp8)
            # three parallel pieces: sync+scalar cover m=0's window fast,
            # the rest rides the otherwise idle gpsimd queue
            nc.sync.dma_start(out=st[:, :, 0:1088], in_=st_r[:, :, 0:1088])
            nc.scalar.dma_start(out=st[:, :, 1088:2304], in_=st_r[:, :, 1088:2304])
            nc.gpsimd.dma_start(out=st[:, :, 2304:N], in_=st_r[:, :, 2304:N])

            with tc.tile_pool(name="mmps", bufs=2, space="PSUM") as mmps:
                for m in range(MT):
                    base = m * P
                    lhs = st[:, :, base : base + P]
                    ot = outp.tile([P, MCOLS], fp8, tag="ot")
                    ps = mmps.tile([P, MCOLS], f32, tag="ps")
                    for q in range(4):
                        c0 = q * 512
                        nc.tensor.matmul(
                            ps[:, c0 : c0 + 512],
                            lhsT=lhs,
                            rhs=st[:, :, base + c0 : base + c0 + 512],
                            perf_mode=DR,
                            start=True,
                            stop=True,
                        )
                    if m % 2 == 0:
                        nc.scalar.copy(out=ot[:], in_=ps[:])
                    else:
                        nc.vector.tensor_copy(out=ot[:], in_=ps[:])
                    nc.sync.dma_start(out=outb[base : base + P, :], in_=ot[:])

                # d=16 tail strip: 16 small matmuls into one more pool tile
                # (rotates into a main slot, overlapping the last copies)
                otc = outp.tile([P, MT * P], fp8, tag="ot")
                pst = mmps.tile([P, MT * P], f32, tag="ps")
                for m in range(MT):
                    base = m * P
                    nc.tensor.matmul(
                        pst[:, base : base + P],
                        lhsT=st[:, :, base : base + P],
                        rhs=st[:, :, base + MCOLS : base + COLS],
                        perf_mode=DR,
                        start=True,
                        stop=True,
                    )
                nc.scalar.copy(out=otc[:], in_=pst[:])
                nc.sync.dma_start(out=outc[:, :], in_=otc[:])

    nc.compile()
    return nc


def _host_prep(prop_state, mask):
    prop = np.asarray(prop_state, dtype=np.float32)
    nrm = np.sqrt(np.einsum("bnd,bnd->bn", prop, prop))
    shat4 = prop * (4.0 / np.maximum(nrm, EPS))[..., None]
    shat4 = shat4.astype(ml_dtypes.float8_e4m3)  # [B, N, D]

    in_maps = []
    for c in range(8):
        b, h = divmod(c, 2)
        r = ROWS * h
        rolled = shat4[b] if r == 0 else np.roll(shat4[b], -r, axis=0)
        # [N, D] -> [P, 2*N] partition-contiguous DoubleRow k-major
        st = np.ascontiguousarray(
            rolled.T.reshape(2, P, N).transpose(1, 0, 2).reshape(P, 2 * N)
        )
        in_maps.append({"st": st})
    return in_maps


def _assemble(results, mask):
    mk = np.asarray(mask)
    out = np.zeros((B, N, N), dtype=np.float32)
    for b in range(B):
        i = mk[b, :, 0].astype(np.int64)
        j = mk[b, :, 1].astype(np.int64)
        mains = [results[2 * b]["outb"], results[2 * b + 1]["outb"]]
        strips = [results[2 * b]["outc"], results[2 * b + 1]["outc"]]
        val = np.empty(E, dtype=np.float64)
        found = np.zeros(E, dtype=bool)
        for x, y in ((i, j), (j, i)):
            for h in (0, 1):
                xr = (x - ROWS * h) % N
                yr = (y - ROWS * h) % N
                cc = yr - (xr >> 7 << 7)
                sel = ~found & (xr < ROWS)
                okm = sel & (cc >= 0) & (cc < MCOLS)
                idx = np.nonzero(okm)[0]
                if idx.size:
                    val[idx] = mains[h][xr[idx], cc[idx]].astype(np.float64)
                    found[idx] = True
                oks = sel & (cc >= MCOLS) & (cc < COLS)
                idx = np.nonzero(oks)[0]
                if idx.size:
                    xi = xr[idx]
                    val[idx] = strips[h][
                        xi & 127, (xi >> 7 << 7) + cc[idx] - MCOLS
                    ].astype(np.float64)
                    found[idx] = True
        assert found.all()
        v = 1.0 / (1.0 + np.exp(-val / 16.0))
        v[i == j] = 0.7310585786300049  # sigmoid(1): self-cossim is exactly 1
        v = v.astype(np.float32)
        out[b, i, j] = v
        out[b, j, i] = v
    return out


def kernel(prop_state, mask):
    from concourse.bass_utils import run_bass_kernel_spmd

    global _prog
    if _prog is None:
        _prog = _build_program()
    in_maps = _host_prep(prop_state, mask)
    res = run_bass_kernel_spmd(_prog, in_maps, core_ids=list(range(8)))
    return _assemble(res.results, mask)
